# revision 6
# baseline (speedup 1.0000x reference)
"""Fused CIN-layer kernel for Trainium2 (8 NeuronCores, batch data-parallel).

True reference semantics (derived from the row-major .view + strided conv):
  out[b, n, c*32+t] = sum_{i<32, y<32} W[n,i,y] * x0[b,t,2i+c] * xk[b,y,2i+c] + bias[n]
where c in {0,1} is the f-parity and i indexes f-pairs.

Per core (128 batches, bc = b_local*2 + c in [0,256), groups J of 4 bc's):
  stage1 (PE):  per i: G_i[n, bc] = sum_y W[n,i,y] * xk[b,y,2i+c]
                i = 4q + r; the 4 r-matmuls of a quad run CONCURRENTLY via
                tile_position=(32r, 0) row tiling (lhsT/rhs live at
                partitions 32r..32r+31).  Concurrent row tiles must drain
                to DISTINCT PSUM banks -> quad tile [64, 2048] fp32 =
                4 banks, each MM writes cols r*512..r*512+256.
                Evac bank-split: ACT r in {0,1}, DVE r in {2,3} ->
                Gsb[n, i*256+bc] fp16 (i-major, contiguous writes).
  transpose (PE): per J: Gt_J[(j,i), n] = Gsb^T via PE transpose (strided
                input AP), fp16 PSUM, evac alternating DVE/ACT.
  stage2 (PE):  out_J[(j,t), n] = sum_{(j,i)} X0bd_J[(j,i),(j,t)] * Gt_J[(j,i),n]
                X0bd = host-built block-diagonal x0 tiles (fp16).
                PSUM fp32 -> fp16 osb (alternating ACT/DVE), DMA out fp16
                in 256KB chunks; host adds bias + final reshape.
  PSUM budget: stage-1 pool (2x4 banks) closes before the gt/po pools
                open, so the stack allocator reuses its banks.
"""

import numpy as np

BS, T, Y, F, NF = 1024, 32, 32, 64, 64
NCORES = 8
BPC = BS // NCORES      # 128 batches per core
NBC = BPC * 2           # 256 (b,c) pairs per core
NG = NBC // 4           # 64 groups of 4
NI = 32                 # f-pair index
NQ = NI // 4            # 8 stage-1 quads

_cached = {}


def _build_bass():
    import concourse.bass as bass
    import concourse.mybir as mybir
    from concourse import bacc
    from concourse.tile import TileContext

    F16 = mybir.dt.float16
    F32 = mybir.dt.float32

    nc = bacc.Bacc()
    # partition (r, y); col (q, bc)
    xks = nc.dram_tensor("xks", [128, NQ * NBC], F16, kind="ExternalInput")
    # partition (r, y); col (q, n)
    wst = nc.dram_tensor("wst", [128, NQ * NF], F16, kind="ExternalInput")
    # block-diagonal x0: partition (j, i); col (J, j2, t)
    x0a = nc.dram_tensor("x0a", [128, NG * 128], F16, kind="ExternalInput")
    iden = nc.dram_tensor("iden", [NF, NF], F16, kind="ExternalInput")
    # out fp16: partition (j, t); col (J, n)
    outd = nc.dram_tensor("outd", [128, NG * NF], F16, kind="ExternalOutput")

    with TileContext(nc) as tc:
        with (
            tc.tile_pool(name="const", bufs=1) as cpool,
            tc.tile_pool(name="sb", bufs=1) as spool,
        ):
            wst_sb = cpool.tile([128, NQ * NF], F16)
            nc.sync.dma_start(out=wst_sb, in_=wst[:, :])
            xks_sb = cpool.tile([128, NQ * NBC], F16)
            h = NQ * NBC // 2
            nc.sync.dma_start(out=xks_sb[:, :h], in_=xks[:, :h])
            nc.sync.dma_start(out=xks_sb[:, h:], in_=xks[:, h:])
            id_sb = cpool.tile([NF, NF], F16)
            nc.sync.dma_start(out=id_sb, in_=iden[:, :])
            x0a_sb = cpool.tile([128, NG * 128], F16)
            hx = NG * 128 // 2
            nc.sync.dma_start(out=x0a_sb[:, :hx], in_=x0a[:, :hx])
            nc.sync.dma_start(out=x0a_sb[:, hx:], in_=x0a[:, hx:])

            gsb = spool.tile([NF, NBC * NI], F16)    # G[n, bc*32+i]
            gt_sb = spool.tile([128, NG * NF], F16)  # Gt[(j,i), J*64+n]
            osb = spool.tile([128, NG * NF], F16)    # out[(j,t), J*64+n]

            # stage 1: 8 quads; quad q covers i = 4q+r with 4 row-tiled
            # concurrent matmuls, one PSUM bank each -> Gsb (i-major)
            with tc.tile_pool(name="gq", bufs=2, space="PSUM") as gqpool:
                for q in range(NQ):
                    gq = gqpool.tile([NF, 4 * 512], mybir.dt.float32, tag="gq")
                    for r in range(4):
                        nc.tensor.matmul(
                            gq[:, r * 512:r * 512 + NBC],
                            wst_sb[32 * r:32 * r + 32, q * NF:(q + 1) * NF],
                            xks_sb[32 * r:32 * r + 32, q * NBC:(q + 1) * NBC],
                            start=True, stop=True,
                            tile_position=(32 * r, 0),
                        )
                    # evac to Gsb[n, bc*32 + 4q + r] (bc-major, so the
                    # transpose input below stays a single free dim);
                    # iteration order (bc, r) on both sides, bank-split on r
                    in_ap = gq[:, :].rearrange(
                        "p (r w) -> p w r", r=4, w=512)[:, :NBC, :]
                    out_ap = gsb[:, :].rearrange(
                        "p (bc i) -> p bc i", bc=NBC, i=NI)[:, :, 4 * q:4 * q + 4]
                    nc.scalar.copy(out_ap[:, :, 0:2], in_ap[:, :, 0:2])         # banks 0-1
                    nc.vector.tensor_copy(out_ap[:, :, 2:4], in_ap[:, :, 2:4])  # banks 2-3

            with (
                tc.tile_pool(name="gt", bufs=2, space="PSUM") as gtpool,
                tc.tile_pool(name="po", bufs=2, space="PSUM") as popool,
            ):
                # transpose: per J: Gsb[n, (j,i)] -> Gt[(j,i), n]
                for J8 in range(NG // 8):
                    gt8 = gtpool.tile([128, 8 * NF], F16, tag="gt8")
                    for s in range(8):
                        J = J8 * 8 + s
                        nc.tensor.transpose(
                            gt8[:, s * NF:(s + 1) * NF],
                            gsb[:, J * 128:(J + 1) * 128],
                            id_sb[:, :],
                        )
                    if J8 % 2 == 0:
                        nc.vector.tensor_copy(
                            gt_sb[:, J8 * 8 * NF:(J8 + 1) * 8 * NF], gt8[:, :])
                    else:
                        nc.scalar.copy(
                            gt_sb[:, J8 * 8 * NF:(J8 + 1) * 8 * NF], gt8[:, :])

                # stage 2: per J, block-diag x0 matmul; fp16 out, chunked DMA
                for J8 in range(NG // 8):
                    po = popool.tile([128, 8 * NF], mybir.dt.float32, tag="po")
                    for s in range(8):
                        J = J8 * 8 + s
                        nc.tensor.matmul(
                            po[:, s * NF:(s + 1) * NF],
                            x0a_sb[:, J * 128:(J + 1) * 128],
                            gt_sb[:, J * NF:(J + 1) * NF],
                            start=True, stop=True,
                        )
                    o0 = J8 * 8 * NF
                    if J8 % 2 == 0:
                        nc.scalar.copy(osb[:, o0:o0 + 8 * NF], po[:, :])
                    else:
                        nc.vector.tensor_copy(osb[:, o0:o0 + 8 * NF], po[:, :])
                    if J8 % 2 == 1:
                        d0 = (J8 - 1) * 8 * NF
                        nc.sync.dma_start(out=outd[:, d0:o0 + 8 * NF],
                                          in_=osb[:, d0:o0 + 8 * NF])
    nc.compile()
    return nc


def _host_prep(x_0, x_k, weight):
    f16 = np.float16
    x_0 = np.asarray(x_0, dtype=np.float32)
    x_k = np.asarray(x_k, dtype=np.float32)
    W = np.asarray(weight, dtype=np.float32).reshape(NF, NI, Y)

    # wst[32r+y, q*64+n] = W[n, 4q+r, y]
    Wr = W.reshape(NF, NQ, 4, Y)                      # n, q, r, y
    wstn = np.ascontiguousarray(
        Wr.transpose(2, 3, 1, 0).reshape(128, NQ * NF)).astype(f16)

    xks_l, x0a_l = [], []
    jj = np.arange(4)
    for core in range(NCORES):
        xkc = x_k[core * BPC:(core + 1) * BPC]        # [128, y, f]
        x0c = x_0[core * BPC:(core + 1) * BPC]        # [128, t, f]
        # xks[32r+y, q*256 + b_l*2 + c] = xk[b_l, y, 2(4q+r)+c]
        xkr = xkc.reshape(BPC, Y, NQ, 4, 2)           # b_l, y, q, r, c
        xksn = xkr.transpose(3, 1, 2, 0, 4).reshape(128, NQ * NBC)
        xks_l.append(np.ascontiguousarray(xksn).astype(f16))
        # x0 per bc: [bc, i, t]
        x0r = x0c.reshape(BPC, T, NI, 2)              # b_l, t, i, c
        x0bc = x0r.transpose(0, 3, 2, 1).reshape(NBC, NI, T)
        # block-diagonal tiles: X0bd[J, j, i, j2, t] = delta(j,j2)*x0bc[4J+j, i, t]
        x0bd = np.zeros((NG, 4, NI, 4, T), dtype=np.float32)
        x0bd[:, jj, :, jj, :] = x0bc.reshape(NG, 4, NI, T).transpose(1, 0, 2, 3)
        # rows (j, i), cols (J, j2, t)
        x0a = x0bd.transpose(1, 2, 0, 3, 4).reshape(128, NG * 128)
        x0a_l.append(np.ascontiguousarray(x0a).astype(f16))

    iden = np.eye(NF, dtype=np.float32).astype(f16)
    return xks_l, x0a_l, wstn, iden


def kernel(x_0, x_k, weight, bias):
    from concourse import bass_utils

    if "nc" not in _cached:
        _cached["nc"] = _build_bass()
    nc = _cached["nc"]

    xks_l, x0a_l, wstn, iden = _host_prep(x_0, x_k, weight)
    in_maps = [
        {"xks": xks_l[c], "x0a": x0a_l[c], "wst": wstn, "iden": iden}
        for c in range(NCORES)
    ]
    res = bass_utils.run_bass_kernel_spmd(nc, in_maps, core_ids=list(range(NCORES)))

    bias = np.asarray(bias, dtype=np.float32)
    outs = []
    for c in range(NCORES):
        od = res.results[c]["outd"].astype(np.float32)  # [128=(j,t), NG*64=(J,n)]
        o = od.reshape(4, T, NG, NF)                # [j, t, J, n]
        o = o.transpose(2, 0, 3, 1)                 # [J, j, n, t]
        o = o.reshape(BPC, 2, NF, T)                # [b_l, c, n, t]
        o = o.transpose(0, 2, 1, 3).reshape(BPC, NF, 2 * T)  # [b_l, n, c*32+t]
        outs.append(o)
    out = np.concatenate(outs, axis=0)
    out = out + bias[None, :, None]
    return np.ascontiguousarray(out.astype(np.float32))


# revision 7
# speedup vs baseline: 1.0775x; 1.0775x over previous
"""Fused CIN-layer kernel for Trainium2 (8 NeuronCores, batch data-parallel).

True reference semantics (derived from the row-major .view + strided conv):
  out[b, n, c*32+t] = sum_{i<32, y<32} W[n,i,y] * x0[b,t,2i+c] * xk[b,y,2i+c] + bias[n]
where c in {0,1} is the f-parity and i indexes f-pairs.

Per core (128 batches, bc = b_local*2 + c in [0,256), groups J of 4 bc's):
  stage1 (PE):  per i: G_i[n, bc] = sum_y W[n,i,y] * xk[b,y,2i+c]
                i = 4q + r; the 4 r-matmuls of a quad run CONCURRENTLY via
                tile_position=(32r, 0) row tiling (lhsT/rhs live at
                partitions 32r..32r+31).  Concurrent row tiles must drain
                to DISTINCT PSUM banks -> quad tile [64, 2048] fp32 =
                4 banks, each MM writes cols r*512..r*512+256.
                Evac bank-split: ACT r in {0,1}, DVE r in {2,3} ->
                Gsb[n, i*256+bc] fp16 (i-major, contiguous writes).
  transpose (PE): per J: Gt_J[(j,i), n] = Gsb^T via PE transpose (strided
                input AP), fp16 PSUM, evac alternating DVE/ACT.
  stage2 (PE):  out_J[(j,t), n] = sum_{(j,i)} X0bd_J[(j,i),(j,t)] * Gt_J[(j,i),n]
                X0bd = host-built block-diagonal x0 tiles (fp16).
                PSUM fp32 -> fp16 osb (alternating ACT/DVE), DMA out fp16
                in 256KB chunks; host adds bias + final reshape.
  PSUM budget: stage-1 pool (2x4 banks) closes before the gt/po pools
                open, so the stack allocator reuses its banks.
"""

import numpy as np

BS, T, Y, F, NF = 1024, 32, 32, 64, 64
NCORES = 8
BPC = BS // NCORES      # 128 batches per core
NBC = BPC * 2           # 256 (b,c) pairs per core
NG = NBC // 4           # 64 groups of 4
NI = 32                 # f-pair index
NQ = NI // 4            # 8 stage-1 quads

_cached = {}


def _build_bass():
    import concourse.bass as bass
    import concourse.mybir as mybir
    from concourse import bacc
    from concourse.tile import TileContext

    F16 = mybir.dt.float16
    F32 = mybir.dt.float32

    nc = bacc.Bacc()
    # partition (r, y); col (q, bc)
    xks = nc.dram_tensor("xks", [128, NQ * NBC], F16, kind="ExternalInput")
    # partition (r, y); col (q, n)
    wst = nc.dram_tensor("wst", [128, NQ * NF], F16, kind="ExternalInput")
    # block-diagonal x0: partition (j, i); col (J, j2, t)
    x0a = nc.dram_tensor("x0a", [128, NG * 128], F16, kind="ExternalInput")
    iden = nc.dram_tensor("iden", [NF, NF], F16, kind="ExternalInput")
    # out fp16: partition (j, t); col (J, n)
    outd = nc.dram_tensor("outd", [128, NG * NF], F16, kind="ExternalOutput")

    with TileContext(nc) as tc:
        with (
            tc.tile_pool(name="const", bufs=1) as cpool,
            tc.tile_pool(name="sb", bufs=1) as spool,
        ):
            wst_sb = cpool.tile([128, NQ * NF], F16)
            nc.sync.dma_start(out=wst_sb, in_=wst[:, :])
            xks_sb = cpool.tile([128, NQ * NBC], F16)
            h = NQ * NBC // 2
            nc.sync.dma_start(out=xks_sb[:, :h], in_=xks[:, :h])
            nc.sync.dma_start(out=xks_sb[:, h:], in_=xks[:, h:])
            id_sb = cpool.tile([NF, NF], F16)
            nc.sync.dma_start(out=id_sb, in_=iden[:, :])
            x0a_sb = cpool.tile([128, NG * 128], F16)
            hx = NG * 128 // 2
            nc.sync.dma_start(out=x0a_sb[:, :hx], in_=x0a[:, :hx])
            nc.sync.dma_start(out=x0a_sb[:, hx:], in_=x0a[:, hx:])

            gsb = spool.tile([NF, NBC * NI], F16)    # G[n, bc*32+i]
            gt_sb = spool.tile([128, NG * NF], F16)  # Gt[(j,i), J*64+n]
            osb = spool.tile([128, NG * NF], F16)    # out[(j,t), J*64+n]

            # stage 1: 8 quads; quad q covers i = 4q+r with 4 row-tiled
            # concurrent matmuls, one PSUM bank each -> Gsb (i-major)
            # Readers of one tile are CHAINED by the Tile scheduler, so the
            # quad PSUM is split into two tiles (gqa: r 0-1, gqb: r 2-3) so
            # the ACT and DVE evacuations run concurrently.  Each matmul
            # still drains to its own PSUM bank (r*512 col offset).
            with tc.tile_pool(name="gq", bufs=2, space="PSUM") as gqpool:
                for q in range(NQ):
                    gqa = gqpool.tile([NF, 2 * 512], mybir.dt.float32, tag="gqa")
                    gqb = gqpool.tile([NF, 2 * 512], mybir.dt.float32, tag="gqb")
                    halves = [gqa, gqa, gqb, gqb]
                    for r in range(4):
                        nc.tensor.matmul(
                            halves[r][:, (r % 2) * 512:(r % 2) * 512 + NBC],
                            wst_sb[32 * r:32 * r + 32, q * NF:(q + 1) * NF],
                            xks_sb[32 * r:32 * r + 32, q * NBC:(q + 1) * NBC],
                            start=True, stop=True,
                            tile_position=(32 * r, 0),
                        )
                    # evac to Gsb[n, bc*32 + 4q + r] (bc-major, so the
                    # transpose input below stays a single free dim);
                    # iteration order (bc, r) on both sides
                    out_ap = gsb[:, :].rearrange(
                        "p (bc i) -> p bc i", bc=NBC, i=NI)[:, :, 4 * q:4 * q + 4]
                    in_a = gqa[:, :].rearrange(
                        "p (r w) -> p w r", r=2, w=512)[:, :NBC, :]
                    in_b = gqb[:, :].rearrange(
                        "p (r w) -> p w r", r=2, w=512)[:, :NBC, :]
                    nc.scalar.copy(out_ap[:, :, 0:2], in_a)
                    nc.vector.tensor_copy(out_ap[:, :, 2:4], in_b)

            with (
                tc.tile_pool(name="gt", bufs=2, space="PSUM") as gtpool,
                tc.tile_pool(name="po", bufs=2, space="PSUM") as popool,
            ):
                # transpose: per J: Gsb[n, (j,i)] -> Gt[(j,i), n]
                for J8 in range(NG // 8):
                    gt8 = gtpool.tile([128, 8 * NF], F16, tag="gt8")
                    for s in range(8):
                        J = J8 * 8 + s
                        nc.tensor.transpose(
                            gt8[:, s * NF:(s + 1) * NF],
                            gsb[:, J * 128:(J + 1) * 128],
                            id_sb[:, :],
                        )
                    if J8 % 2 == 0:
                        nc.vector.tensor_copy(
                            gt_sb[:, J8 * 8 * NF:(J8 + 1) * 8 * NF], gt8[:, :])
                    else:
                        nc.scalar.copy(
                            gt_sb[:, J8 * 8 * NF:(J8 + 1) * 8 * NF], gt8[:, :])

                # stage 2: per J, block-diag x0 matmul; fp16 out, chunked DMA
                for J8 in range(NG // 8):
                    po = popool.tile([128, 8 * NF], mybir.dt.float32, tag="po")
                    for s in range(8):
                        J = J8 * 8 + s
                        nc.tensor.matmul(
                            po[:, s * NF:(s + 1) * NF],
                            x0a_sb[:, J * 128:(J + 1) * 128],
                            gt_sb[:, J * NF:(J + 1) * NF],
                            start=True, stop=True,
                        )
                    o0 = J8 * 8 * NF
                    if J8 % 2 == 0:
                        nc.scalar.copy(osb[:, o0:o0 + 8 * NF], po[:, :])
                    else:
                        nc.vector.tensor_copy(osb[:, o0:o0 + 8 * NF], po[:, :])
                    if J8 % 2 == 1:
                        d0 = (J8 - 1) * 8 * NF
                        nc.sync.dma_start(out=outd[:, d0:o0 + 8 * NF],
                                          in_=osb[:, d0:o0 + 8 * NF])
    nc.compile()
    return nc


def _host_prep(x_0, x_k, weight):
    f16 = np.float16
    x_0 = np.asarray(x_0, dtype=np.float32)
    x_k = np.asarray(x_k, dtype=np.float32)
    W = np.asarray(weight, dtype=np.float32).reshape(NF, NI, Y)

    # wst[32r+y, q*64+n] = W[n, 4q+r, y]
    Wr = W.reshape(NF, NQ, 4, Y)                      # n, q, r, y
    wstn = np.ascontiguousarray(
        Wr.transpose(2, 3, 1, 0).reshape(128, NQ * NF)).astype(f16)

    xks_l, x0a_l = [], []
    jj = np.arange(4)
    for core in range(NCORES):
        xkc = x_k[core * BPC:(core + 1) * BPC]        # [128, y, f]
        x0c = x_0[core * BPC:(core + 1) * BPC]        # [128, t, f]
        # xks[32r+y, q*256 + b_l*2 + c] = xk[b_l, y, 2(4q+r)+c]
        xkr = xkc.reshape(BPC, Y, NQ, 4, 2)           # b_l, y, q, r, c
        xksn = xkr.transpose(3, 1, 2, 0, 4).reshape(128, NQ * NBC)
        xks_l.append(np.ascontiguousarray(xksn).astype(f16))
        # x0 per bc: [bc, i, t]
        x0r = x0c.reshape(BPC, T, NI, 2)              # b_l, t, i, c
        x0bc = x0r.transpose(0, 3, 2, 1).reshape(NBC, NI, T)
        # block-diagonal tiles: X0bd[J, j, i, j2, t] = delta(j,j2)*x0bc[4J+j, i, t]
        x0bd = np.zeros((NG, 4, NI, 4, T), dtype=np.float32)
        x0bd[:, jj, :, jj, :] = x0bc.reshape(NG, 4, NI, T).transpose(1, 0, 2, 3)
        # rows (j, i), cols (J, j2, t)
        x0a = x0bd.transpose(1, 2, 0, 3, 4).reshape(128, NG * 128)
        x0a_l.append(np.ascontiguousarray(x0a).astype(f16))

    iden = np.eye(NF, dtype=np.float32).astype(f16)
    return xks_l, x0a_l, wstn, iden


def kernel(x_0, x_k, weight, bias):
    from concourse import bass_utils

    if "nc" not in _cached:
        _cached["nc"] = _build_bass()
    nc = _cached["nc"]

    xks_l, x0a_l, wstn, iden = _host_prep(x_0, x_k, weight)
    in_maps = [
        {"xks": xks_l[c], "x0a": x0a_l[c], "wst": wstn, "iden": iden}
        for c in range(NCORES)
    ]
    res = bass_utils.run_bass_kernel_spmd(nc, in_maps, core_ids=list(range(NCORES)))

    bias = np.asarray(bias, dtype=np.float32)
    outs = []
    for c in range(NCORES):
        od = res.results[c]["outd"].astype(np.float32)  # [128=(j,t), NG*64=(J,n)]
        o = od.reshape(4, T, NG, NF)                # [j, t, J, n]
        o = o.transpose(2, 0, 3, 1)                 # [J, j, n, t]
        o = o.reshape(BPC, 2, NF, T)                # [b_l, c, n, t]
        o = o.transpose(0, 2, 1, 3).reshape(BPC, NF, 2 * T)  # [b_l, n, c*32+t]
        outs.append(o)
    out = np.concatenate(outs, axis=0)
    out = out + bias[None, :, None]
    return np.ascontiguousarray(out.astype(np.float32))
